# revision 1
# baseline (speedup 1.0000x reference)
"""Cosine-similarity retrieval kernel for Trainium2 (Bass/Tile, 8 NeuronCores).

Computes sims[i] = dot(word_vectors[i], q) / ||word_vectors[i]|| with
q = inputs / ||inputs|| (query normalization folded in on the host).

Sharding: word_vectors row-sharded across 8 cores, query broadcast.
Each core processes R = 25088 rows (196 tiles of 128 rows); core 7's
slice overlaps core 6's by 704 rows so every core runs the identical
program (one NEFF), and the overlap rows compute bitwise-identical
values.

Per-core dataflow (memory-bound; HBM floor ~= 102.8 MB / ~360 GB/s):
  - rows are mapped to SBUF via the interleave  row = p*T + t
    (partition p in [0,128), tile t in [0,T)), so both the W loads and
    the final sims store are plain strided DMAs - no transpose anywhere.
  - per 128-row tile: one DVE tensor_tensor_reduce (elementwise mult
    with broadcast q + free-dim add-reduce, single 1x pass) for the dot,
    and one ACT activation(Square, accum_out) pass for the squared norm.
  - epilogue: norm = sqrt(norm2) (ACT), inv = 1/norm (DVE iterative
    divide), sims = dots * inv (DVE), one DMA out.
"""

import numpy as np

D = 1024          # embedding dim
N_FULL = 200000   # total rows
NCORES = 8
R = 25088         # rows per core = 128 * 196
T = R // 128      # 196 column-tiles per core
NT = 4            # tiles per DMA chunk (2 MiB per dma_start)
NCHUNK = T // NT  # 49

_NC_CACHE = {}


def _build_nc():
    if "nc" in _NC_CACHE:
        return _NC_CACHE["nc"]

    import concourse.tile as tile
    from concourse import bacc, mybir

    fp32 = mybir.dt.float32
    nc = bacc.Bacc(
        "TRN2",
        target_bir_lowering=False,
        debug=False,
        enable_asserts=False,
        num_devices=NCORES,
        enable_partition_id=False,
    )
    w = nc.dram_tensor("w", [R, D], fp32, kind="ExternalInput").ap()
    q = nc.dram_tensor("q", [D], fp32, kind="ExternalInput").ap()
    out = nc.dram_tensor("out", [R], fp32, kind="ExternalOutput").ap()

    # row p*T + t  <->  SBUF partition p, tile-column t
    w_v = w.rearrange("(p t) d -> p (t d)", p=128)  # [128, T*D], 4KB*T contig/part
    out_v = out.rearrange("(p t) -> p t", p=128)    # [128, T]

    with tile.TileContext(nc) as tc:
        with (
            tc.tile_pool(name="win", bufs=4) as win_pool,
            tc.tile_pool(name="aux", bufs=1) as aux_pool,
        ):
            qb = aux_pool.tile([128, D], fp32)
            nc.sync.dma_start(qb, q.partition_broadcast(128))

            dots = aux_pool.tile([128, T], fp32)
            norm2 = aux_pool.tile([128, T], fp32)
            scr_v = aux_pool.tile([128, D], fp32)
            scr_a = aux_pool.tile([128, D], fp32)

            for c in range(NCHUNK):
                wt = win_pool.tile([128, NT * D], fp32, name="wt")
                nc.sync.dma_start(wt, w_v[:, c * NT * D : (c + 1) * NT * D])
                for j in range(NT):
                    t = c * NT + j
                    sl = wt[:, j * D : (j + 1) * D]
                    # fused dot: scr = (sl*1+0)*qb, dots[:,t] = sum(scr).
                    # (TENSOR_TENSOR_REDUCE crashes this runtime; the
                    # custom-DVE affine_mul_reduce is the working fused
                    # multiply+reduce at the same 1x streaming rate.)
                    nc.vector.affine_mul_reduce(
                        out=scr_v,
                        accum_out=dots[:, t : t + 1],
                        in0=sl,
                        in1=qb,
                        scale=1.0,
                        bias=0.0,
                    )
                    nc.scalar.activation(
                        out=scr_a,
                        in_=sl,
                        func=mybir.ActivationFunctionType.Square,
                        accum_out=norm2[:, t : t + 1],
                    )

            norm = aux_pool.tile([128, T], fp32)
            nc.scalar.sqrt(norm, norm2)
            inv = aux_pool.tile([128, T], fp32)
            nc.vector.reciprocal(inv, norm)
            sims = aux_pool.tile([128, T], fp32)
            nc.vector.tensor_mul(sims, dots, inv)
            nc.sync.dma_start(out_v, sims)

    nc.compile()
    _NC_CACHE["nc"] = nc
    return nc


def _shard_starts():
    starts = [i * R for i in range(NCORES - 1)]
    starts.append(N_FULL - R)  # core 7 overlaps core 6 by 704 rows
    return starts


def make_in_maps(inputs: np.ndarray, word_vectors: np.ndarray):
    inputs = np.ascontiguousarray(inputs, dtype=np.float32)
    word_vectors = np.ascontiguousarray(word_vectors, dtype=np.float32)
    qn = inputs / np.maximum(np.linalg.norm(inputs), np.float32(1e-12))
    qn = qn.astype(np.float32)
    return [
        {"w": word_vectors[s : s + R], "q": qn} for s in _shard_starts()
    ]


def assemble(results) -> np.ndarray:
    full = np.empty(N_FULL, dtype=np.float32)
    for s, res in zip(_shard_starts(), results):
        full[s : s + R] = res["out"]
    return full


def kernel(inputs: np.ndarray, word_vectors: np.ndarray) -> np.ndarray:
    from concourse import bass_utils

    nc = _build_nc()
    in_maps = make_in_maps(inputs, word_vectors)
    res = bass_utils.run_bass_kernel_spmd(
        nc, in_maps, core_ids=list(range(NCORES))
    )
    return assemble(res.results)



# revision 2
# speedup vs baseline: 227.6445x; 227.6445x over previous
"""Cosine-similarity retrieval kernel for Trainium2 (Bass/Tile, 8 NeuronCores).

Computes sims[i] = dot(word_vectors[i], q) / ||word_vectors[i]|| with
q = inputs / ||inputs|| (query normalization folded in on the host).

Sharding: word_vectors row-sharded across 8 cores, query broadcast.
Each core processes R = 25088 rows (196 tiles of 128 rows); core 7's
slice overlaps core 6's by 704 rows so every core runs the identical
program (one NEFF).

W is staged to bf16 on the host (one-time input re-encode; rel-err
budget is 2e-2, bf16 staging costs ~1e-3). This halves the per-core
HBM traffic vs fp32: 51.4 MB/core, DMA floor ~150 us at the measured
~345 GB/s. Engine rates are dtype-insensitive here (measured):
DVE custom dot pass 1225+80 ns/tile, ACT square 1148+279 ns/tile, so
post-bf16 the kernel is ACT/DVE-compute-bound at ~270-285 us/core
(vs ~347 us max-core for the fp32 version, which was DMA-bound).

Per-core dataflow:
  - rows are mapped to SBUF via the interleave  row = p*T + t
    (partition p in [0,128), tile t in [0,T)), so W loads and the sims
    store are plain strided DMAs - no transpose anywhere. Chunks of
    NT=8 tiles keep 16 KB contiguous per partition line per dma_start.
  - per 128-row tile: one DVE affine_mul_reduce (w*q multiply + free-dim
    add-reduce) for the dot, one ACT activation(Square, accum_out) for
    the squared norm. A few tiles' squares run on DVE instead of ACT to
    balance the two engines (s_v below).
  - epilogue: norm = sqrt(norm2) (ACT), inv = 1/norm (DVE), sims =
    dots * inv (DVE), one DMA out.
"""

import numpy as np

D = 1024          # embedding dim
N_FULL = 200000   # total rows
NCORES = 8
R = 25088         # rows per core = 128 * 196
T = R // 128      # 196 column-tiles per core
NT = 8            # tiles per DMA chunk (16KB/partition line in bf16)
SQ_ON_DVE = 3     # of every 32 tiles, this many squares go to DVE

_NC_CACHE = {}


def _build_nc():
    if "nc" in _NC_CACHE:
        return _NC_CACHE["nc"]

    import concourse.tile as tile
    from concourse import bacc, mybir

    fp32 = mybir.dt.float32
    bf16 = mybir.dt.bfloat16
    nc = bacc.Bacc(
        "TRN2",
        target_bir_lowering=False,
        debug=False,
        enable_asserts=False,
        num_devices=NCORES,
        enable_partition_id=False,
    )
    w = nc.dram_tensor("w", [R, D], bf16, kind="ExternalInput").ap()
    q = nc.dram_tensor("q", [D], bf16, kind="ExternalInput").ap()
    out = nc.dram_tensor("out", [R], fp32, kind="ExternalOutput").ap()

    # row p*T + t  <->  SBUF partition p, tile-column t
    w_v = w.rearrange("(p t) d -> p (t d)", p=128)  # [128, T*D]
    out_v = out.rearrange("(p t) -> p t", p=128)    # [128, T]

    # chunk schedule: 24 chunks of 8 tiles + 1 chunk of 4 tiles = 196
    chunks = [(c * NT, NT) for c in range(24)] + [(192, 4)]

    with tile.TileContext(nc) as tc:
        with (
            tc.tile_pool(name="win", bufs=5) as win_pool,
            tc.tile_pool(name="aux", bufs=1) as aux_pool,
        ):
            qb = aux_pool.tile([128, D], bf16)
            nc.sync.dma_start(qb, q.partition_broadcast(128))

            dots = aux_pool.tile([128, T], fp32)
            norm2 = aux_pool.tile([128, T], fp32)
            scr_v = aux_pool.tile([128, D], bf16)
            scr_a = aux_pool.tile([128, D], bf16)

            for s, n in chunks:
                wt = win_pool.tile([128, NT * D], bf16, name="wt")
                nc.sync.dma_start(
                    wt[:, : n * D], w_v[:, s * D : (s + n) * D]
                )
                for j in range(n):
                    t = s + j
                    sl = wt[:, j * D : (j + 1) * D]
                    # dot: scr = (sl*1+0)*qb, dots[:,t] = sum(scr)
                    nc.vector.affine_mul_reduce(
                        out=scr_v,
                        accum_out=dots[:, t : t + 1],
                        in0=sl,
                        in1=qb,
                        scale=1.0,
                        bias=0.0,
                    )
                    if t % 32 < SQ_ON_DVE:
                        # balance: a few squares on DVE (same fused op)
                        nc.vector.affine_mul_reduce(
                            out=scr_v,
                            accum_out=norm2[:, t : t + 1],
                            in0=sl,
                            in1=sl,
                            scale=1.0,
                            bias=0.0,
                        )
                    else:
                        nc.scalar.activation(
                            out=scr_a,
                            in_=sl,
                            func=mybir.ActivationFunctionType.Square,
                            accum_out=norm2[:, t : t + 1],
                        )

            norm = aux_pool.tile([128, T], fp32)
            nc.scalar.sqrt(norm, norm2)
            inv = aux_pool.tile([128, T], fp32)
            nc.vector.reciprocal(inv, norm)
            sims = aux_pool.tile([128, T], fp32)
            nc.vector.tensor_mul(sims, dots, inv)
            nc.sync.dma_start(out_v, sims)

    nc.compile()
    _NC_CACHE["nc"] = nc
    return nc


def _shard_starts():
    starts = [i * R for i in range(NCORES - 1)]
    starts.append(N_FULL - R)  # core 7 overlaps core 6 by 704 rows
    return starts


def make_in_maps(inputs: np.ndarray, word_vectors: np.ndarray):
    import ml_dtypes

    inputs = np.ascontiguousarray(inputs, dtype=np.float32)
    word_vectors = np.ascontiguousarray(word_vectors, dtype=np.float32)
    qn = inputs / np.maximum(np.linalg.norm(inputs), np.float32(1e-12))
    qn16 = qn.astype(ml_dtypes.bfloat16)
    w16 = word_vectors.astype(ml_dtypes.bfloat16)
    return [{"w": w16[s : s + R], "q": qn16} for s in _shard_starts()]


def assemble(results) -> np.ndarray:
    full = np.empty(N_FULL, dtype=np.float32)
    for s, res in zip(_shard_starts(), results):
        full[s : s + R] = res["out"]
    return full


def kernel(inputs: np.ndarray, word_vectors: np.ndarray) -> np.ndarray:
    from concourse import bass_utils

    nc = _build_nc()
    in_maps = make_in_maps(inputs, word_vectors)
    res = bass_utils.run_bass_kernel_spmd(
        nc, in_maps, core_ids=list(range(NCORES))
    )
    return assemble(res.results)
